# revision 3
# baseline (speedup 1.0000x reference)
"""Contrastive loss (GRACE-style) on 8 Trainium2 NeuronCores.

loss = sum_i 0.5*(l1_i + l2_i)
  l1 = -log(diag(exp(h1@h2.T/t)) / (rowsum(exp(h1@h1.T/t)) + rowsum(exp(h1@h2.T/t)) - diag(exp(h1@h1.T/t))))
  l2 = same with h1<->h2;  h = z / ||z||_row,  t = 0.2

Sharding: columns (j) of the similarity matrices are sharded across 8 cores
(each core owns a 1024-column chunk of both h1 and h2). Each core computes,
for ALL 8192 rows i, the partial sums over its j-chunk of
exp(s_i * (z_i . h_j) / t), where h_j is the normalized chunk column and the
row normalization s_i/t is applied inside the ACT exp via a per-partition
scale vector. refl and between column blocks are concatenated into one
[512, 2048] rhs so one ACT instruction accumulates refl+between partial
row-sums together. Host sums the 8 partials in fp64 and applies logs.
"""

import numpy as np
import ml_dtypes

N = 8192
D = 512
NCORES = 8
CH = N // NCORES  # 1024 columns per core
P = 128
KD = D // P  # 4 contraction tiles
NIB = N // P  # 64 i-blocks
NCT = CH // P  # 8 chunk row-tiles
NZC = 8  # zt column chunks (of 1024) per kd tile
TAU_INV = 5.0

_CACHE = {}


def _build():
    import concourse.tile as tile
    from concourse import bacc, mybir
    from concourse.masks import make_identity

    f32 = mybir.dt.float32
    bf16 = mybir.dt.bfloat16
    AF = mybir.ActivationFunctionType
    ALU = mybir.AluOpType

    nc = bacc.Bacc("TRN2", target_bir_lowering=False, debug=False,
                   num_devices=NCORES)

    z1t = nc.dram_tensor("z1t", [D, N], bf16, kind="ExternalInput")
    z2t = nc.dram_tensor("z2t", [D, N], bf16, kind="ExternalInput")
    z1r = nc.dram_tensor("z1r", [N, D], bf16, kind="ExternalInput")
    z2r = nc.dram_tensor("z2r", [N, D], bf16, kind="ExternalInput")
    z1c = nc.dram_tensor("z1c", [CH, D], bf16, kind="ExternalInput")
    z2c = nc.dram_tensor("z2c", [CH, D], bf16, kind="ExternalInput")
    partials = nc.dram_tensor("partials", [2, N], f32, kind="ExternalOutput")
    diag = nc.dram_tensor("diag", [CH], f32, kind="ExternalOutput")

    z1t_v = z1t.rearrange("(k p) n -> p k n", p=P)
    z2t_v = z2t.rearrange("(k p) n -> p k n", p=P)

    with tile.TileContext(nc) as tc:
        with (
            tc.tile_pool(name="singles", bufs=1) as singles,
            tc.tile_pool(name="zr", bufs=4) as zrp,
            tc.tile_pool(name="scr", bufs=3) as scrp,
            tc.tile_pool(name="h", bufs=3) as hp,
            tc.tile_pool(name="es", bufs=2) as esp,
            tc.tile_pool(name="ps", bufs=2, space="PSUM") as psp,
        ):
            ident = singles.tile([P, P], bf16, tag="ident")
            make_identity(nc, ident)

            # ---- persistent buffers ----
            # stationary operands, 64 chunk tiles [128, 1024]
            zt_tiles = {}
            for nm in ("zt1", "zt2"):
                zt_tiles[nm] = [
                    [singles.tile([P, CH], bf16, tag=f"{nm}_{kd}_{c}",
                                  name=f"{nm}_{kd}_{c}")
                     for c in range(NZC)]
                    for kd in range(KD)
                ]
            rhs = singles.tile([P, KD, 2 * CH], bf16, tag="rhs")
            sq1 = singles.tile([P, NIB], f32, tag="sq1")
            sq2 = singles.tile([P, NIB], f32, tag="sq2")
            s1tau = singles.tile([P, NIB], f32, tag="s1tau")
            s2tau = singles.tile([P, NIB], f32, tag="s2tau")
            stmp = singles.tile([P, NIB], f32, tag="stmp")
            stmp2 = singles.tile([P, NIB], f32, tag="stmp2")
            acc1 = singles.tile([P, NIB], f32, tag="acc1")
            acc2 = singles.tile([P, NIB], f32, tag="acc2")
            sqc = singles.tile([P, 2 * NCT], f32, tag="sqc")
            sctmp = singles.tile([P, 2 * NCT], f32, tag="sctmp")
            sc = singles.tile([P, 2 * NCT], f32, tag="sc")
            dotc = singles.tile([P, NCT], f32, tag="dotc")
            v5a = singles.tile([P, NCT], f32, tag="v5a")
            v5 = singles.tile([P, NCT], f32, tag="v5")
            c1_tiles = [singles.tile([P, D], bf16, tag=f"c1_{t}",
                                     name=f"c1_{t}") for t in range(NCT)]
            c2_tiles = [singles.tile([P, D], bf16, tag=f"c2_{t}",
                                     name=f"c2_{t}") for t in range(NCT)]

            # ---- DMA: chunk rows first (gate everything local) ----
            for t in range(NCT):
                nc.sync.dma_start(out=c1_tiles[t],
                                  in_=z1c[t * P:(t + 1) * P, :])
                nc.sync.dma_start(out=c2_tiles[t],
                                  in_=z2c[t * P:(t + 1) * P, :])

            # zt1 first chunks so pass-1 matmuls can start early
            for c in range(2):
                for kd in range(KD):
                    nc.sync.dma_start(
                        out=zt_tiles["zt1"][kd][c],
                        in_=z1t_v[:, kd, c * CH:(c + 1) * CH])

            # ---- chunk norms + cross dots (DVE) ----
            for t in range(NCT):
                s = scrp.tile([P, D], f32, tag="scr")
                nc.vector.tensor_mul(s, c1_tiles[t], c1_tiles[t])
                nc.vector.tensor_reduce(sqc[:, t:t + 1], s,
                                        axis=mybir.AxisListType.X, op=ALU.add)
                s = scrp.tile([P, D], f32, tag="scr")
                nc.vector.tensor_mul(s, c2_tiles[t], c2_tiles[t])
                nc.vector.tensor_reduce(sqc[:, NCT + t:NCT + t + 1], s,
                                        axis=mybir.AxisListType.X, op=ALU.add)
                s = scrp.tile([P, D], f32, tag="scr")
                nc.vector.tensor_mul(s, c1_tiles[t], c2_tiles[t])
                nc.vector.tensor_reduce(dotc[:, t:t + 1], s,
                                        axis=mybir.AxisListType.X, op=ALU.add)

            # sc = 1/sqrt(sqc)
            nc.scalar.activation(out=sctmp, in_=sqc, func=AF.Sqrt)
            nc.vector.reciprocal(sc, sctmp)

            # v5 = dotc * sc1 * sc2 * (1/tau)  ( = ln between_ii for own rows)
            nc.vector.tensor_mul(v5a, dotc, sc[:, 0:NCT])
            nc.vector.tensor_mul(v5a, v5a, sc[:, NCT:2 * NCT])
            nc.vector.tensor_scalar_mul(v5, v5a, TAU_INV)
            nc.sync.dma_start(out=diag.rearrange("(t p) -> p t", p=P), in_=v5)

            # ---- normalize chunk + transpose into rhs ----
            for t in range(NCT):
                for half, (ct, scol) in enumerate(
                        [(c1_tiles[t], t), (c2_tiles[t], NCT + t)]):
                    h = hp.tile([P, D], bf16, tag="h")
                    nc.scalar.mul(h, ct, sc[:, scol:scol + 1])
                    for kd in range(KD):
                        pst = psp.tile([P, P], bf16, tag="ps")
                        nc.tensor.transpose(pst, h[:, kd * P:(kd + 1) * P],
                                            ident)
                        nc.vector.tensor_copy(
                            rhs[:, kd, half * CH + t * P: half * CH + (t + 1) * P],
                            pst)

            # ---- full row norms: z1 (needed before first exp) ----
            for t in range(NIB):
                zr = zrp.tile([P, D], bf16, tag="zr")
                nc.sync.dma_start(out=zr, in_=z1r[t * P:(t + 1) * P, :])
                s = scrp.tile([P, D], f32, tag="scr")
                nc.vector.tensor_mul(s, zr, zr)
                nc.vector.tensor_reduce(sq1[:, t:t + 1], s,
                                        axis=mybir.AxisListType.X, op=ALU.add)
            nc.scalar.activation(out=stmp, in_=sq1, func=AF.Sqrt)
            nc.vector.reciprocal(stmp, stmp)
            nc.vector.tensor_scalar_mul(s1tau, stmp, TAU_INV)

            # rest of zt1
            for c in range(2, NZC):
                for kd in range(KD):
                    nc.sync.dma_start(
                        out=zt_tiles["zt1"][kd][c],
                        in_=z1t_v[:, kd, c * CH:(c + 1) * CH])

            # z2 row norms (needed before pass 2 exps)
            for t in range(NIB):
                zr = zrp.tile([P, D], bf16, tag="zr")
                nc.sync.dma_start(out=zr, in_=z2r[t * P:(t + 1) * P, :])
                s = scrp.tile([P, D], f32, tag="scr")
                nc.vector.tensor_mul(s, zr, zr)
                nc.vector.tensor_reduce(sq2[:, t:t + 1], s,
                                        axis=mybir.AxisListType.X, op=ALU.add)
            nc.scalar.activation(out=stmp2, in_=sq2, func=AF.Sqrt)
            nc.vector.reciprocal(stmp2, stmp2)
            nc.vector.tensor_scalar_mul(s2tau, stmp2, TAU_INV)

            # zt2
            for c in range(NZC):
                for kd in range(KD):
                    nc.sync.dma_start(
                        out=zt_tiles["zt2"][kd][c],
                        in_=z2t_v[:, kd, c * CH:(c + 1) * CH])

            # ---- main: 2 passes x 64 i-blocks ----
            for nm, stau, acc in (("zt1", s1tau, acc1), ("zt2", s2tau, acc2)):
                for ib in range(NIB):
                    c, lb = divmod(ib, NZC)
                    ps = psp.tile([P, 4 * 512], f32, tag="ps")
                    for kd in range(KD):
                        lhsT = zt_tiles[nm][kd][c][:, lb * P:(lb + 1) * P]
                        for jt in range(4):
                            nc.tensor.matmul(
                                ps[:, jt * 512:(jt + 1) * 512],
                                lhsT=lhsT,
                                rhs=rhs[:, kd, jt * 512:(jt + 1) * 512],
                                start=(kd == 0),
                                stop=(kd == KD - 1),
                            )
                    es = esp.tile([P, 4 * 512], bf16, tag="es")
                    nc.scalar.activation(
                        out=es, in_=ps, func=AF.Exp,
                        scale=stau[:, ib:ib + 1],
                        accum_out=acc[:, ib:ib + 1],
                    )

            nc.sync.dma_start(
                out=partials[0].rearrange("(b p) -> p b", p=P), in_=acc1)
            nc.sync.dma_start(
                out=partials[1].rearrange("(b p) -> p b", p=P), in_=acc2)

    nc.compile()
    return nc


def _get_nc():
    if "nc" not in _CACHE:
        _CACHE["nc"] = _build()
    return _CACHE["nc"]


def kernel(z1, z2):
    from concourse.bass_utils import run_bass_kernel_spmd

    z1 = np.asarray(z1, dtype=np.float32)
    z2 = np.asarray(z2, dtype=np.float32)
    bf16 = ml_dtypes.bfloat16

    z1r = np.ascontiguousarray(z1.astype(bf16))
    z2r = np.ascontiguousarray(z2.astype(bf16))
    z1t = np.ascontiguousarray(z1r.T)
    z2t = np.ascontiguousarray(z2r.T)

    in_maps = []
    for r in range(NCORES):
        in_maps.append({
            "z1t": z1t, "z2t": z2t, "z1r": z1r, "z2r": z2r,
            "z1c": np.ascontiguousarray(z1r[r * CH:(r + 1) * CH]),
            "z2c": np.ascontiguousarray(z2r[r * CH:(r + 1) * CH]),
        })

    nc = _get_nc()
    res = run_bass_kernel_spmd(nc, in_maps, core_ids=list(range(NCORES)))

    S1 = np.zeros(N, dtype=np.float64)
    S2 = np.zeros(N, dtype=np.float64)
    v5 = np.zeros(N, dtype=np.float64)
    for r in range(NCORES):
        out = res.results[r]
        S1 += out["partials"][0].astype(np.float64)
        S2 += out["partials"][1].astype(np.float64)
        v5[r * CH:(r + 1) * CH] = out["diag"].astype(np.float64)

    e5 = np.exp(np.float64(TAU_INV))
    loss = 0.5 * (np.log(S1 - e5) + np.log(S2 - e5)) - v5
    return np.float32(loss.sum())
